# revision 25
# baseline (speedup 1.0000x reference)
"""Trainium2 Bass kernel for nn_Net_73710228734901.

The network's post-gather graph (concat -> Conv3d -> spatial mean -> Linear)
is entirely linear in the gathered pixels, and the gathers / avg-pool /
1x1-conv are linear in the inputs.  Since the output is only [B, 1], the
whole model collapses to

    out[b] = lin_b + <W1, x1crop[b]> + <W2, x2crop[b]> + <W4, sharecrop[b]>
                   + <W3, x3[b]>

with fixed weight tensors computed (cheaply, on host) from c_w / conv3d_w /
lin_w / idx_h / idx_w.  The _genetic gather reads only a 7x7 window per
channel of x1/x2/share, so only those 49 of 196 pixels per channel carry
nonzero weight -- the host ships exactly those windows to the device
(pure index selection, no arithmetic).  Per (partition, batch) the device
reduces F = 3*49 + 980 + 1pad = 1128 elements.

Device kernel (per core, channel-sharded; DMA-bound at ~400 GB/s):
  - x streams in 4-batch blocks [128, 4, 1128] fp16 on the sync HWDGE
    ring; the single [128, 1128] weight tile rides the scalar ring.
  - DVE: one tensor_tensor multiply per block (weights broadcast via a
    stride-0 AP; fp16 in/out -> 2x DVE mode, ~2.5us per 4 batches).
  - The free-dim reduction is split across the two otherwise-idle
    engines, alternating per block:
      * PE blocks: 9 ones-matmuls of 512 flat columns (batch boundaries
        ignored) accumulate chunk column-sums into one PSUM bank; the
        one-hot row selector is a sliding window over a ones-column
        buffer, so no per-batch stationary tensors are DMA'd.  The host
        untangles the [81, 512] chunk sums (pure reshaping).
      * ACT blocks: per-batch activation(Copy) with fp32 accum_out.
  - first/last blocks run batch-at-a-time to shorten the ramp and tail.
Host combines the per-core partials, un-scales, adds lin_b.

Sharding: channels 8 ways (x1/x2/share: 128 ch/core, x3: 160 ch/core);
every core sees all 64 batches; per-core HBM traffic 18.5 MB.
"""

import numpy as np

import concourse.bacc as bacc
import concourse.mybir as mybir
from concourse.bass_utils import run_bass_kernel_spmd
from concourse.tile import TileContext

NCORES = 8
NB = 64            # full batch, all on every core (channel sharding)
FC = 49            # cropped 7x7 window per channel (x1/x2/share)
F3 = 980           # x3 shard: 160 ch * 784 pos / 128 partitions
F_TOT = 3 * FC + F3 + 1   # 1128 (zero pad col -> even, 4B-aligned rows)
BLK = 4            # batches per DMA block / per DVE multiply
XBUFS = 8          # x-tile buffer depth
NBLK = NB // BLK   # 16
PE_BLOCKS = (0, 1, 4, 6, 8, 10, 12, 14, 15)  # 9 blocks -> TensorE path
ACT_BLOCKS = (2, 3, 5, 7, 9, 11, 13)         # 7 blocks -> ScalarE path
SINGLES = (0, 14, 15)  # batch-at-a-time blocks (short ramp + short tail)
NCHUNK = 9         # ceil(BLK*F_TOT / 512) flat 512-col chunks per PE block
CHW = 512          # chunk width = one PSUM bank row
PADF = NCHUNK * CHW - BLK * F_TOT   # 96 zero cols at the end of each prod
W_SCALE = 1024.0   # weights pre-scaled by 2^10 so fp16 products avoid
                   # subnormals; undone exactly in the final host combine

_F32 = mybir.dt.float32
_F16 = mybir.dt.float16


def _build_fold(c_w, conv3d_w, lin_w, idx_h, idx_w):
    """Collapse conv3d+mean+linear into per-element weights (float64 host).

    Returns A1, A2, A4: [1024, 49] crop-window weights for x1/x2/share,
    and Ws3: [1280, 784] full-grid weights for x3.
    """
    c_w = c_w.astype(np.float64)
    conv3d_w = conv3d_w.astype(np.float64)
    lin_w = lin_w.astype(np.float64)

    # W2[c = i*64+dd, kh, kw] = sum_{o,d,kd: 3d-4+kd=dd} lin_w[o*24+d]
    #                                  * conv3d_w[o,i,kd,kh,kw]
    W2 = np.zeros((1024, 3, 3), np.float64)
    o_idx = np.arange(32) * 24
    i_idx = np.arange(16) * 64
    for d in range(24):
        for kd in range(3):
            dd = 3 * d - 4 + kd
            if 0 <= dd < 64:
                W2[i_idx + dd] += np.einsum(
                    'o,oikl->ikl', lin_w[o_idx + d, 0], conv3d_w[:, :, kd])

    # Mean over the 14x14 conv output folds each (kh,kw) tap into a
    # border mask.
    M = np.zeros((3, 3, 14, 14), np.float64)
    rng = {0: (0, 13), 1: (0, 14), 2: (1, 14)}
    for kh in range(3):
        for kw in range(3):
            r0, r1 = rng[kh]
            c0, c1 = rng[kw]
            M[kh, kw, r0:r1, c0:c1] = 1.0
    A = np.einsum('ckl,klrs->crs', W2, M) / 196.0   # [1024, 14, 14]

    # Quadrants of the 14x14 concat grid: rows<7,cols<7 = g1(x1);
    # rows>=7,cols<7 = g2(x2); rows<7,cols>=7 = g3(x3 path);
    # rows>=7,cols>=7 = gs(share).  g1/g2/gs weights apply directly to the
    # 7x7 crop windows; only the x3 path needs the scatter (c_w mixes
    # channels with different crop offsets).
    A1 = A[:, 0:7, 0:7].reshape(1024, 49)
    A2 = A[:, 7:14, 0:7].reshape(1024, 49)
    A4 = A[:, 7:14, 7:14].reshape(1024, 49)

    A3 = A[:, 0:7, 7:14]
    Ws3c = np.zeros((1024, 14, 14), np.float64)
    ci = np.arange(1024)[:, None, None]
    ri = (idx_h[2][:, None] + np.arange(7))[:, :, None]
    wi = (idx_w[2][:, None] + np.arange(7))[:, None, :]
    Ws3c[ci, ri, wi] = A3

    # Pull the scattered weights back through the 1x1 conv ...
    Wpool = np.einsum('oc,ohw->chw', c_w, Ws3c)     # [1280, 14, 14]
    # ... and through avg_pool2d(5, stride 2, pad 2) (transposed scatter).
    Ws3 = np.zeros((1280, 28, 28), np.float64)
    for dh in range(-2, 3):
        for dw in range(-2, 3):
            hs = [h for h in range(14) if 0 <= 2 * h + dh < 28]
            ws = [w for w in range(14) if 0 <= 2 * w + dw < 28]
            H = [2 * h + dh for h in hs]
            W_ = [2 * w + dw for w in ws]
            Ws3[:, np.ix_(H, W_)[0], np.ix_(H, W_)[1]] += \
                Wpool[:, np.ix_(hs, ws)[0], np.ix_(hs, ws)[1]] / 25.0

    return (A1.astype(np.float32), A2.astype(np.float32),
            A4.astype(np.float32), Ws3.reshape(1280, 784).astype(np.float32))


def _crop(x, ih, iw):
    """Gather the per-channel 7x7 crop windows: [B,1024,14,14] -> [B,1024,49]."""
    B = x.shape[0]
    ci = np.arange(1024)[:, None, None]
    ri = (ih[:, None] + np.arange(7))[:, :, None]
    wi = (iw[:, None] + np.arange(7))[:, None, :]
    return x[:, ci, ri, wi].reshape(B, 1024, 49).astype(np.float16)


def _act_cols():
    """Batch id for each dense ACT accumulator column."""
    return [blk * BLK + j for blk in ACT_BLOCKS for j in range(BLK)]


def _flat_segs(j):
    """Split batch j's flat range [j*F, (j+1)*F) at CHW boundaries.

    Yields (fs, fe, row, col): batch-local f range -> (psum row, col).
    """
    lo, hi = j * F_TOT, (j + 1) * F_TOT
    while lo < hi:
        nxt = min(hi, (lo // CHW + 1) * CHW)
        yield (lo - j * F_TOT, nxt - j * F_TOT, lo // CHW, lo % CHW)
        lo = nxt


def _build_bass(xbufs=XBUFS):
    nc = bacc.Bacc("TRN2")
    n_act = len(ACT_BLOCKS) * BLK                       # 28
    n_rows = len(PE_BLOCKS) * NCHUNK                    # 81 psum rows
    xin = nc.dram_tensor("xin", [128, NB, F_TOT], _F16, kind="ExternalInput")
    win = nc.dram_tensor("win", [128, F_TOT], _F16, kind="ExternalInput")
    outp = nc.dram_tensor("outp", [n_rows, CHW], _F32, kind="ExternalOutput")
    outa = nc.dram_tensor("outa", [1, n_act], _F32, kind="ExternalOutput")

    with TileContext(nc) as tc:
        with (
            tc.tile_pool(name="xpool", bufs=xbufs) as xpool,
            tc.tile_pool(name="gpool", bufs=3) as gpool,
            tc.tile_pool(name="apool", bufs=1) as apool,
            tc.tile_pool(name="ppool", bufs=1, space="PSUM") as ppool,
        ):
            cpool = apool
            wt = cpool.tile([128, F_TOT], _F16)
            nc.scalar.dma_start(out=wt[:], in_=win[:, :])
            wbb = wt[:].unsqueeze(1).broadcast_to([128, BLK, F_TOT])

            # Sliding ones-column window: z[:, 128] = 1, else 0.  The
            # stationary for psum row r is z[:, 128-r : 256-r] (col r of
            # that window is the ones column).
            z = cpool.tile([128, 256], _F16)
            nc.gpsimd.memset(z[:], 0.0)
            nc.gpsimd.memset(z[:, 128:129], 1.0)
            ones32 = cpool.tile([128, 1], _F32)
            nc.gpsimd.memset(ones32[:], 1.0)

            pchunk = ppool.tile([128, CHW], _F32)       # PE chunk sums
            psa = ppool.tile([1, n_act], _F32)          # ACT batch sums
            acc = apool.tile([128, n_act], _F32)        # ACT accum columns

            first_mm = [True]

            def emit_mm(out_ap, lhsT, rhs, last=False):
                nc.tensor.matmul(out_ap, lhsT=lhsT, rhs=rhs,
                                 start=first_mm[0], stop=last)
                first_mm[0] = False

            pe_i = 0            # dense PE-block index
            act_i = 0           # dense ACT column base
            blocked_i = 0       # physical prod-buffer rotation counter
            for k in range(NBLK):
                single = k in SINGLES
                # Alternate the two HWDGE rings so the core can pull at
                # full fabric rate when its HBM-stack partner is idle.
                ring = nc.sync if k % 2 == 0 else nc.scalar
                if single:
                    prods = []
                    for j in range(BLK):
                        xt1 = xpool.tile([128, F_TOT], _F16, tag="xt1")
                        (nc.sync if j % 2 == 0 else nc.scalar).dma_start(
                            out=xt1[:], in_=xin[:, k * BLK + j, :])
                        prod1 = gpool.tile([128, F_TOT], _F16, tag="prod1",
                                           bufs=4)
                        nc.vector.tensor_tensor(
                            prod1[:], xt1[:], wt[:], mybir.AluOpType.mult)
                        prods.append(prod1)
                else:
                    xt = xpool.tile([128, BLK, F_TOT], _F16, tag="xt")
                    ring.dma_start(
                        out=xt[:], in_=xin[:, k * BLK:(k + 1) * BLK, :])
                    prod = gpool.tile([128, BLK * F_TOT + PADF], _F16,
                                      tag="prod", bufs=4)
                    if blocked_i < 4:
                        # zero the 96 flat pad cols once per physical
                        # buffer (4-deep rotation; TT never writes them):
                        # they enter the chunk-8 matmul, and NaNs there
                        # would poison whole psum columns.
                        nc.gpsimd.memset(prod[:, BLK * F_TOT:], 0.0)
                    blocked_i += 1
                    nc.vector.tensor_tensor(
                        prod[:, 0:BLK * F_TOT], xt[:], wbb,
                        mybir.AluOpType.mult)

                if k in PE_BLOCKS:
                    lastk = (k == PE_BLOCKS[-1])
                    if single:
                        # per-batch prod tiles: emit CHW-aligned segment
                        # matmuls that land in the same flat rows/cols as
                        # the blocked chunk layout.
                        segs = [(j,) + s for j in range(BLK)
                                for s in _flat_segs(j)]
                        for i, (j, fs, fe, row, col) in enumerate(segs):
                            r = pe_i * NCHUNK + row
                            emit_mm(pchunk[:, col:col + (fe - fs)],
                                    z[:, 128 - r:256 - r],
                                    prods[j][:, fs:fe],
                                    last=(lastk and i == len(segs) - 1))
                    else:
                        for c in range(NCHUNK):
                            r = pe_i * NCHUNK + c
                            emit_mm(pchunk[:, :],
                                    z[:, 128 - r:256 - r],
                                    prod[:, c * CHW:(c + 1) * CHW],
                                    last=(lastk and c == NCHUNK - 1))
                    pe_i += 1
                else:
                    for j in range(BLK):
                        src = (prods[j][:] if single
                               else prod[:, j * F_TOT:(j + 1) * F_TOT])
                        sink = gpool.tile([128, F_TOT], _F16, tag="sink")
                        nc.scalar.activation(
                            sink[:], src,
                            mybir.ActivationFunctionType.Copy,
                            accum_out=acc[:, act_i:act_i + 1])
                        act_i += 1

            # ACT partition-sum: ones-matmul over the dense accum columns.
            nc.tensor.matmul(psa[:], lhsT=ones32[:], rhs=acc[:],
                             start=True, stop=True)
            resa = apool.tile([1, n_act], _F32)
            nc.vector.tensor_copy(resa[:], psa[:])
            nc.scalar.dma_start(out=outa[:, :], in_=resa[:])

            # PE chunk sums -> SBUF -> DRAM (host finishes the reduction).
            # Sync ring: independent of outa's scalar-ring FIFO.
            resp = apool.tile([n_rows, CHW], _F32)
            nc.vector.tensor_copy(resp[:], pchunk[0:n_rows, :])
            nc.sync.dma_start(out=outp[:, :], in_=resp[:])
    nc.finalize()
    return nc


def _shard_inputs(x1, x2, x3, share_feature, A1, A2, A4, Ws3,
                  idx_h, idx_w):
    """Host-side layout: crop-gather + channel-shard + fp16 cast."""
    x1c = _crop(np.asarray(x1), idx_h[0], idx_w[0])       # [64,1024,49] f16
    x2c = _crop(np.asarray(x2), idx_h[1], idx_w[1])
    shc = _crop(np.asarray(share_feature), idx_h[3], idx_w[3])
    x3h = np.asarray(x3, dtype=np.float16).reshape(NB, 1280 * 784)

    in_maps = []
    for m in range(NCORES):
        cs = slice(m * 128, (m + 1) * 128)
        x3f = x3h[:, m * 128 * F3:(m + 1) * 128 * F3].reshape(NB, 128, F3)
        xin = np.concatenate([
            x1c[:, cs], x2c[:, cs], shc[:, cs], x3f,
            np.zeros((NB, 128, 1), np.float16),
        ], axis=2)                                    # [64, 128, 1128]
        xin = np.ascontiguousarray(xin.transpose(1, 0, 2))  # [128, 64, 1128]

        win = np.concatenate([
            A1[cs], A2[cs], A4[cs],
            Ws3.reshape(-1)[m * 128 * F3:(m + 1) * 128 * F3].reshape(128, F3),
            np.zeros((128, 1), np.float32),
        ], axis=1) * W_SCALE                          # [128, 1128]
        in_maps.append({'xin': xin, 'win': win.astype(np.float16)})
    return in_maps


def _combine(results, lin_b):
    """Sum per-core partials; untangle PE chunk rows; add bias."""
    total = np.zeros(NB, np.float64)
    for r in results:
        p = r['outp'].astype(np.float64)          # [81, 512]
        a = r['outa'].astype(np.float64).reshape(-1)   # [28]
        flat = p.reshape(len(PE_BLOCKS), NCHUNK * CHW)[:, :BLK * F_TOT]
        dots = flat.reshape(len(PE_BLOCKS), BLK, F_TOT).sum(axis=2)
        for i, blk in enumerate(PE_BLOCKS):
            total[blk * BLK:(blk + 1) * BLK] += dots[i]
        for i, b in enumerate(_act_cols()):
            total[b] += a[i]
    return total / W_SCALE + float(lin_b[0])


def _ensure_ntff_hook():
    """Make `trace=True` (e.g. BASS_TRACE=1) work under axon even when the
    image's antenv package lacks axon_hooks: register an equivalent module
    backed by the ctypes NTFF hook from trn_agent_boot."""
    import sys
    import types
    try:
        import antenv.axon_hooks  # noqa: F401
        return
    except Exception:
        pass
    try:
        from trn_agent_boot import trn_boot
        hook = trn_boot._ntff_profile_via_ctypes('/opt/axon/libaxon_pjrt.so')
        mod = types.ModuleType('antenv.axon_hooks')
        mod.get_axon_ntff_profile_hook = lambda: hook
        mod.set_axon_ntff_profile_hook = lambda h: None
        sys.modules['antenv.axon_hooks'] = mod
    except Exception:
        pass


def kernel(x1, x2, x3, share_feature, c_w, conv3d_w, lin_w, lin_b,
           idx_h, idx_w):
    x1, x2, x3 = np.asarray(x1), np.asarray(x2), np.asarray(x3)
    share_feature = np.asarray(share_feature)
    c_w, conv3d_w = np.asarray(c_w), np.asarray(conv3d_w)
    lin_w, lin_b = np.asarray(lin_w), np.asarray(lin_b)
    idx_h, idx_w = np.asarray(idx_h), np.asarray(idx_w)
    _ensure_ntff_hook()
    A1, A2, A4, Ws3 = _build_fold(c_w, conv3d_w, lin_w, idx_h, idx_w)
    in_maps = _shard_inputs(x1, x2, x3, share_feature,
                            A1, A2, A4, Ws3, idx_h, idx_w)
    nc = _build_bass()
    res = run_bass_kernel_spmd(nc, in_maps, core_ids=list(range(NCORES)))
    return _combine(res.results, lin_b).astype(np.float32).reshape(NB, 1)


# revision 27
# speedup vs baseline: 1.1173x; 1.1173x over previous
"""Trainium2 Bass kernel for nn_Net_73710228734901.

The network's post-gather graph (concat -> Conv3d -> spatial mean -> Linear)
is entirely linear in the gathered pixels, and the gathers / avg-pool /
1x1-conv are linear in the inputs.  Since the output is only [B, 1], the
whole model collapses to

    out[b] = lin_b + <W1, x1crop[b]> + <W2, x2crop[b]> + <W4, sharecrop[b]>
                   + <W3, x3[b]>

with fixed weight tensors computed (cheaply, on host) from c_w / conv3d_w /
lin_w / idx_h / idx_w.  The _genetic gather reads only a 7x7 window per
channel of x1/x2/share, so only those 49 of 196 pixels per channel carry
nonzero weight -- the host ships exactly those windows to the device
(pure index selection, no arithmetic).  Per (partition, batch) the device
reduces F = 3*49 + 980 + 1pad = 1128 elements.

Device kernel (per core, channel-sharded; DMA-bound at ~400 GB/s):
  - x streams in 4-batch blocks [128, 4, 1128] fp16 on the sync HWDGE
    ring; the single [128, 1128] weight tile rides the scalar ring.
  - DVE: one tensor_tensor multiply per block (weights broadcast via a
    stride-0 AP; fp16 in/out -> 2x DVE mode, ~2.5us per 4 batches).
  - The free-dim reduction is split across the two otherwise-idle
    engines, alternating per block:
      * PE blocks: 9 ones-matmuls of 512 flat columns (batch boundaries
        ignored) accumulate chunk column-sums into one PSUM bank; the
        one-hot row selector is a sliding window over a ones-column
        buffer, so no per-batch stationary tensors are DMA'd.  The host
        untangles the [81, 512] chunk sums (pure reshaping).
      * ACT blocks: per-batch activation(Copy) with fp32 accum_out.
  - first/last blocks run batch-at-a-time to shorten the ramp and tail.
Host combines the per-core partials, un-scales, adds lin_b.

Sharding: channels 8 ways (x1/x2/share: 128 ch/core, x3: 160 ch/core);
every core sees all 64 batches; per-core HBM traffic 18.5 MB.
"""

import numpy as np

import concourse.bacc as bacc
import concourse.mybir as mybir
from concourse.bass_utils import run_bass_kernel_spmd
from concourse.tile import TileContext

NCORES = 8
NB = 64            # full batch, all on every core (channel sharding)
FC = 49            # cropped 7x7 window per channel (x1/x2/share)
F3 = 980           # x3 shard: 160 ch * 784 pos / 128 partitions
F_TOT = 3 * FC + F3 + 1   # 1128 (zero pad col -> even, 4B-aligned rows)
BLK = 4            # batches per DMA block / per DVE multiply
XBUFS = 6          # x-tile buffer depth
NBLK = NB // BLK   # 16
PE_BLOCKS = (0, 1, 4, 6, 8, 10, 12, 14, 15)  # 9 blocks -> TensorE path
ACT_BLOCKS = (2, 3, 5, 7, 9, 11, 13)         # 7 blocks -> ScalarE path
SINGLES = (0, 14, 15)  # batch-at-a-time blocks (short ramp + short tail)
NCHUNK = 9         # ceil(BLK*F_TOT / 512) flat 512-col chunks per PE block
CHW = 512          # chunk width = one PSUM bank row
PADF = NCHUNK * CHW - BLK * F_TOT   # 96 zero cols at the end of each prod
W_SCALE = 1024.0   # weights pre-scaled by 2^10 so fp16 products avoid
                   # subnormals; undone exactly in the final host combine

_F32 = mybir.dt.float32
_F16 = mybir.dt.float16


def _build_fold(c_w, conv3d_w, lin_w, idx_h, idx_w):
    """Collapse conv3d+mean+linear into per-element weights (float64 host).

    Returns A1, A2, A4: [1024, 49] crop-window weights for x1/x2/share,
    and Ws3: [1280, 784] full-grid weights for x3.
    """
    c_w = c_w.astype(np.float64)
    conv3d_w = conv3d_w.astype(np.float64)
    lin_w = lin_w.astype(np.float64)

    # W2[c = i*64+dd, kh, kw] = sum_{o,d,kd: 3d-4+kd=dd} lin_w[o*24+d]
    #                                  * conv3d_w[o,i,kd,kh,kw]
    W2 = np.zeros((1024, 3, 3), np.float64)
    o_idx = np.arange(32) * 24
    i_idx = np.arange(16) * 64
    for d in range(24):
        for kd in range(3):
            dd = 3 * d - 4 + kd
            if 0 <= dd < 64:
                W2[i_idx + dd] += np.einsum(
                    'o,oikl->ikl', lin_w[o_idx + d, 0], conv3d_w[:, :, kd])

    # Mean over the 14x14 conv output folds each (kh,kw) tap into a
    # border mask.
    M = np.zeros((3, 3, 14, 14), np.float64)
    rng = {0: (0, 13), 1: (0, 14), 2: (1, 14)}
    for kh in range(3):
        for kw in range(3):
            r0, r1 = rng[kh]
            c0, c1 = rng[kw]
            M[kh, kw, r0:r1, c0:c1] = 1.0
    A = np.einsum('ckl,klrs->crs', W2, M) / 196.0   # [1024, 14, 14]

    # Quadrants of the 14x14 concat grid: rows<7,cols<7 = g1(x1);
    # rows>=7,cols<7 = g2(x2); rows<7,cols>=7 = g3(x3 path);
    # rows>=7,cols>=7 = gs(share).  g1/g2/gs weights apply directly to the
    # 7x7 crop windows; only the x3 path needs the scatter (c_w mixes
    # channels with different crop offsets).
    A1 = A[:, 0:7, 0:7].reshape(1024, 49)
    A2 = A[:, 7:14, 0:7].reshape(1024, 49)
    A4 = A[:, 7:14, 7:14].reshape(1024, 49)

    A3 = A[:, 0:7, 7:14]
    Ws3c = np.zeros((1024, 14, 14), np.float64)
    ci = np.arange(1024)[:, None, None]
    ri = (idx_h[2][:, None] + np.arange(7))[:, :, None]
    wi = (idx_w[2][:, None] + np.arange(7))[:, None, :]
    Ws3c[ci, ri, wi] = A3

    # Pull the scattered weights back through the 1x1 conv ...
    Wpool = np.einsum('oc,ohw->chw', c_w, Ws3c)     # [1280, 14, 14]
    # ... and through avg_pool2d(5, stride 2, pad 2) (transposed scatter).
    Ws3 = np.zeros((1280, 28, 28), np.float64)
    for dh in range(-2, 3):
        for dw in range(-2, 3):
            hs = [h for h in range(14) if 0 <= 2 * h + dh < 28]
            ws = [w for w in range(14) if 0 <= 2 * w + dw < 28]
            H = [2 * h + dh for h in hs]
            W_ = [2 * w + dw for w in ws]
            Ws3[:, np.ix_(H, W_)[0], np.ix_(H, W_)[1]] += \
                Wpool[:, np.ix_(hs, ws)[0], np.ix_(hs, ws)[1]] / 25.0

    return (A1.astype(np.float32), A2.astype(np.float32),
            A4.astype(np.float32), Ws3.reshape(1280, 784).astype(np.float32))


def _crop(x, ih, iw):
    """Gather the per-channel 7x7 crop windows: [B,1024,14,14] -> [B,1024,49]."""
    B = x.shape[0]
    ci = np.arange(1024)[:, None, None]
    ri = (ih[:, None] + np.arange(7))[:, :, None]
    wi = (iw[:, None] + np.arange(7))[:, None, :]
    return x[:, ci, ri, wi].reshape(B, 1024, 49).astype(np.float16)


def _act_cols():
    """Batch id for each dense ACT accumulator column."""
    return [blk * BLK + j for blk in ACT_BLOCKS for j in range(BLK)]


def _flat_segs(j):
    """Split batch j's flat range [j*F, (j+1)*F) at CHW boundaries.

    Yields (fs, fe, row, col): batch-local f range -> (psum row, col).
    """
    lo, hi = j * F_TOT, (j + 1) * F_TOT
    while lo < hi:
        nxt = min(hi, (lo // CHW + 1) * CHW)
        yield (lo - j * F_TOT, nxt - j * F_TOT, lo // CHW, lo % CHW)
        lo = nxt


def _build_bass(xbufs=XBUFS):
    nc = bacc.Bacc("TRN2")
    n_act = len(ACT_BLOCKS) * BLK                       # 28
    n_rows = len(PE_BLOCKS) * NCHUNK                    # 81 psum rows
    xin = nc.dram_tensor("xin", [128, NB, F_TOT], _F16, kind="ExternalInput")
    win = nc.dram_tensor("win", [128, F_TOT], _F16, kind="ExternalInput")
    outp = nc.dram_tensor("outp", [n_rows, CHW], _F32, kind="ExternalOutput")
    outa = nc.dram_tensor("outa", [1, n_act], _F32, kind="ExternalOutput")

    with TileContext(nc) as tc:
        with (
            tc.tile_pool(name="xpool", bufs=xbufs) as xpool,
            tc.tile_pool(name="gpool", bufs=3) as gpool,
            tc.tile_pool(name="apool", bufs=1) as apool,
            tc.tile_pool(name="ppool", bufs=1, space="PSUM") as ppool,
        ):
            cpool = apool
            wt = cpool.tile([128, F_TOT], _F16)
            nc.scalar.dma_start(out=wt[:], in_=win[:, :])
            wbb = wt[:].unsqueeze(1).broadcast_to([128, BLK, F_TOT])

            # Sliding ones-column window: z[:, 128] = 1, else 0.  The
            # stationary for psum row r is z[:, 128-r : 256-r] (col r of
            # that window is the ones column).
            z = cpool.tile([128, 256], _F16)
            nc.gpsimd.memset(z[:], 0.0)
            nc.gpsimd.memset(z[:, 128:129], 1.0)
            ones32 = cpool.tile([128, 1], _F32)
            nc.gpsimd.memset(ones32[:], 1.0)

            pchunk = ppool.tile([128, CHW], _F32)       # PE chunk sums
            psa = ppool.tile([1, n_act], _F32)          # ACT batch sums
            acc = apool.tile([128, n_act], _F32)        # ACT accum columns

            first_mm = [True]

            def emit_mm(out_ap, lhsT, rhs, last=False):
                nc.tensor.matmul(out_ap, lhsT=lhsT, rhs=rhs,
                                 start=first_mm[0], stop=last)
                first_mm[0] = False

            pe_i = 0            # dense PE-block index
            act_i = 0           # dense ACT column base
            blocked_i = 0       # physical prod-buffer rotation counter
            for k in range(NBLK):
                single = k in SINGLES
                if single:
                    prods = []
                    for j in range(BLK):
                        xt1 = xpool.tile([128, F_TOT], _F16, tag="xt1")
                        nc.sync.dma_start(
                            out=xt1[:], in_=xin[:, k * BLK + j, :])
                        prod1 = gpool.tile([128, F_TOT], _F16, tag="prod1",
                                           bufs=4)
                        nc.vector.tensor_tensor(
                            prod1[:], xt1[:], wt[:], mybir.AluOpType.mult)
                        prods.append(prod1)
                else:
                    xt = xpool.tile([128, BLK, F_TOT], _F16, tag="xt")
                    nc.sync.dma_start(
                        out=xt[:], in_=xin[:, k * BLK:(k + 1) * BLK, :])
                    prod = gpool.tile([128, BLK * F_TOT + PADF], _F16,
                                      tag="prod", bufs=4)
                    if blocked_i < 4:
                        # zero the 96 flat pad cols once per physical
                        # buffer (4-deep rotation; TT never writes them):
                        # they enter the chunk-8 matmul, and NaNs there
                        # would poison whole psum columns.
                        nc.gpsimd.memset(prod[:, BLK * F_TOT:], 0.0)
                    blocked_i += 1
                    nc.vector.tensor_tensor(
                        prod[:, 0:BLK * F_TOT], xt[:], wbb,
                        mybir.AluOpType.mult)

                if k in PE_BLOCKS:
                    lastk = (k == PE_BLOCKS[-1])
                    if single:
                        # per-batch prod tiles: emit CHW-aligned segment
                        # matmuls that land in the same flat rows/cols as
                        # the blocked chunk layout.
                        segs = [(j,) + s for j in range(BLK)
                                for s in _flat_segs(j)]
                        for i, (j, fs, fe, row, col) in enumerate(segs):
                            r = pe_i * NCHUNK + row
                            emit_mm(pchunk[:, col:col + (fe - fs)],
                                    z[:, 128 - r:256 - r],
                                    prods[j][:, fs:fe],
                                    last=(lastk and i == len(segs) - 1))
                    else:
                        for c in range(NCHUNK):
                            r = pe_i * NCHUNK + c
                            emit_mm(pchunk[:, :],
                                    z[:, 128 - r:256 - r],
                                    prod[:, c * CHW:(c + 1) * CHW],
                                    last=(lastk and c == NCHUNK - 1))
                    pe_i += 1
                else:
                    for j in range(BLK):
                        src = (prods[j][:] if single
                               else prod[:, j * F_TOT:(j + 1) * F_TOT])
                        sink = gpool.tile([128, F_TOT], _F16, tag="sink")
                        nc.scalar.activation(
                            sink[:], src,
                            mybir.ActivationFunctionType.Copy,
                            accum_out=acc[:, act_i:act_i + 1])
                        act_i += 1

            # ACT partition-sum: ones-matmul over the dense accum columns.
            nc.tensor.matmul(psa[:], lhsT=ones32[:], rhs=acc[:],
                             start=True, stop=True)
            resa = apool.tile([1, n_act], _F32)
            nc.vector.tensor_copy(resa[:], psa[:])
            nc.scalar.dma_start(out=outa[:, :], in_=resa[:])

            # PE chunk sums -> SBUF -> DRAM (host finishes the reduction).
            # Sync ring: independent of outa's scalar-ring FIFO.
            resp = apool.tile([n_rows, CHW], _F32)
            nc.vector.tensor_copy(resp[:], pchunk[0:n_rows, :])
            nc.sync.dma_start(out=outp[:, :], in_=resp[:])
    nc.finalize()
    return nc


def _shard_inputs(x1, x2, x3, share_feature, A1, A2, A4, Ws3,
                  idx_h, idx_w):
    """Host-side layout: crop-gather + channel-shard + fp16 cast."""
    x1c = _crop(np.asarray(x1), idx_h[0], idx_w[0])       # [64,1024,49] f16
    x2c = _crop(np.asarray(x2), idx_h[1], idx_w[1])
    shc = _crop(np.asarray(share_feature), idx_h[3], idx_w[3])
    x3h = np.asarray(x3, dtype=np.float16).reshape(NB, 1280 * 784)

    in_maps = []
    for m in range(NCORES):
        cs = slice(m * 128, (m + 1) * 128)
        x3f = x3h[:, m * 128 * F3:(m + 1) * 128 * F3].reshape(NB, 128, F3)
        xin = np.concatenate([
            x1c[:, cs], x2c[:, cs], shc[:, cs], x3f,
            np.zeros((NB, 128, 1), np.float16),
        ], axis=2)                                    # [64, 128, 1128]
        xin = np.ascontiguousarray(xin.transpose(1, 0, 2))  # [128, 64, 1128]

        win = np.concatenate([
            A1[cs], A2[cs], A4[cs],
            Ws3.reshape(-1)[m * 128 * F3:(m + 1) * 128 * F3].reshape(128, F3),
            np.zeros((128, 1), np.float32),
        ], axis=1) * W_SCALE                          # [128, 1128]
        in_maps.append({'xin': xin, 'win': win.astype(np.float16)})
    return in_maps


def _combine(results, lin_b):
    """Sum per-core partials; untangle PE chunk rows; add bias."""
    total = np.zeros(NB, np.float64)
    for r in results:
        p = r['outp'].astype(np.float64)          # [81, 512]
        a = r['outa'].astype(np.float64).reshape(-1)   # [28]
        flat = p.reshape(len(PE_BLOCKS), NCHUNK * CHW)[:, :BLK * F_TOT]
        dots = flat.reshape(len(PE_BLOCKS), BLK, F_TOT).sum(axis=2)
        for i, blk in enumerate(PE_BLOCKS):
            total[blk * BLK:(blk + 1) * BLK] += dots[i]
        for i, b in enumerate(_act_cols()):
            total[b] += a[i]
    return total / W_SCALE + float(lin_b[0])


def _ensure_ntff_hook():
    """Make `trace=True` (e.g. BASS_TRACE=1) work under axon even when the
    image's antenv package lacks axon_hooks: register an equivalent module
    backed by the ctypes NTFF hook from trn_agent_boot."""
    import sys
    import types
    try:
        import antenv.axon_hooks  # noqa: F401
        return
    except Exception:
        pass
    try:
        from trn_agent_boot import trn_boot
        hook = trn_boot._ntff_profile_via_ctypes('/opt/axon/libaxon_pjrt.so')
        mod = types.ModuleType('antenv.axon_hooks')
        mod.get_axon_ntff_profile_hook = lambda: hook
        mod.set_axon_ntff_profile_hook = lambda h: None
        sys.modules['antenv.axon_hooks'] = mod
    except Exception:
        pass


def kernel(x1, x2, x3, share_feature, c_w, conv3d_w, lin_w, lin_b,
           idx_h, idx_w):
    x1, x2, x3 = np.asarray(x1), np.asarray(x2), np.asarray(x3)
    share_feature = np.asarray(share_feature)
    c_w, conv3d_w = np.asarray(c_w), np.asarray(conv3d_w)
    lin_w, lin_b = np.asarray(lin_w), np.asarray(lin_b)
    idx_h, idx_w = np.asarray(idx_h), np.asarray(idx_w)
    _ensure_ntff_hook()
    A1, A2, A4, Ws3 = _build_fold(c_w, conv3d_w, lin_w, idx_h, idx_w)
    in_maps = _shard_inputs(x1, x2, x3, share_feature,
                            A1, A2, A4, Ws3, idx_h, idx_w)
    nc = _build_bass()
    res = run_bass_kernel_spmd(nc, in_maps, core_ids=list(range(NCORES)))
    return _combine(res.results, lin_b).astype(np.float32).reshape(NB, 1)


# revision 29
# speedup vs baseline: 1.1512x; 1.0303x over previous
"""Trainium2 Bass kernel for nn_Net_73710228734901.

The network's post-gather graph (concat -> Conv3d -> spatial mean -> Linear)
is entirely linear in the gathered pixels, and the gathers / avg-pool /
1x1-conv are linear in the inputs.  Since the output is only [B, 1], the
whole model collapses to

    out[b] = lin_b + <W1, x1crop[b]> + <W2, x2crop[b]> + <W4, sharecrop[b]>
                   + <W3, x3[b]>

with fixed weight tensors computed (cheaply, on host) from c_w / conv3d_w /
lin_w / idx_h / idx_w.  The _genetic gather reads only a 7x7 window per
channel of x1/x2/share, so only those 49 of 196 pixels per channel carry
nonzero weight -- the host ships exactly those windows to the device
(pure index selection, no arithmetic).  Per (partition, batch) the device
reduces F = 3*49 + 980 + 1pad = 1128 elements.

Device kernel (per core, channel-sharded; DMA-bound at ~400 GB/s):
  - x streams in 4-batch blocks [128, 4, 1128] fp16 on the sync HWDGE
    ring; the single [128, 1128] weight tile rides the scalar ring.
  - DVE: one tensor_tensor multiply per block (weights broadcast via a
    stride-0 AP; fp16 in/out -> 2x DVE mode, ~2.5us per 4 batches).
  - The free-dim reduction is split across the two otherwise-idle
    engines, alternating per block:
      * PE blocks: 9 ones-matmuls of 512 flat columns (batch boundaries
        ignored) accumulate chunk column-sums into one PSUM bank; the
        one-hot row selector is a sliding window over a ones-column
        buffer, so no per-batch stationary tensors are DMA'd.  The host
        untangles the [81, 512] chunk sums (pure reshaping).
      * ACT blocks: per-batch activation(Copy) with fp32 accum_out.
  - first/last blocks run batch-at-a-time to shorten the ramp and tail.
Host combines the per-core partials, un-scales, adds lin_b.

Sharding: channels 8 ways (x1/x2/share: 128 ch/core, x3: 160 ch/core);
every core sees all 64 batches; per-core HBM traffic 18.5 MB.
"""

import numpy as np

import concourse.bacc as bacc
import concourse.mybir as mybir
from concourse.bass_utils import run_bass_kernel_spmd
from concourse.tile import TileContext

NCORES = 8
NB = 64            # full batch, all on every core (channel sharding)
FC = 49            # cropped 7x7 window per channel (x1/x2/share)
F3 = 980           # x3 shard: 160 ch * 784 pos / 128 partitions
F_TOT = 3 * FC + F3 + 1   # 1128 (zero pad col -> even, 4B-aligned rows)
BLK = 4            # batches per DMA block / per DVE multiply
XBUFS = 8          # x-tile buffer depth (deep: absorbs DMA-completion
                   # semaphore jitter without stalling the TT cadence)
NBLK = NB // BLK   # 16
PE_BLOCKS = (0, 1, 4, 6, 8, 10, 12, 14, 15)  # 9 blocks -> TensorE path
ACT_BLOCKS = (2, 3, 5, 7, 9, 11, 13)         # 7 blocks -> ScalarE path
SINGLES = (0, 14, 15)  # batch-at-a-time blocks (short ramp + short tail)
NCHUNK = 9         # ceil(BLK*F_TOT / 512) flat 512-col chunks per PE block
CHW = 512          # chunk width = one PSUM bank row
PADF = NCHUNK * CHW - BLK * F_TOT   # 96 zero cols at the end of each prod
W_SCALE = 1024.0   # weights pre-scaled by 2^10 so fp16 products avoid
                   # subnormals; undone exactly in the final host combine

_F32 = mybir.dt.float32
_F16 = mybir.dt.float16


def _build_fold(c_w, conv3d_w, lin_w, idx_h, idx_w):
    """Collapse conv3d+mean+linear into per-element weights (float64 host).

    Returns A1, A2, A4: [1024, 49] crop-window weights for x1/x2/share,
    and Ws3: [1280, 784] full-grid weights for x3.
    """
    c_w = c_w.astype(np.float64)
    conv3d_w = conv3d_w.astype(np.float64)
    lin_w = lin_w.astype(np.float64)

    # W2[c = i*64+dd, kh, kw] = sum_{o,d,kd: 3d-4+kd=dd} lin_w[o*24+d]
    #                                  * conv3d_w[o,i,kd,kh,kw]
    W2 = np.zeros((1024, 3, 3), np.float64)
    o_idx = np.arange(32) * 24
    i_idx = np.arange(16) * 64
    for d in range(24):
        for kd in range(3):
            dd = 3 * d - 4 + kd
            if 0 <= dd < 64:
                W2[i_idx + dd] += np.einsum(
                    'o,oikl->ikl', lin_w[o_idx + d, 0], conv3d_w[:, :, kd])

    # Mean over the 14x14 conv output folds each (kh,kw) tap into a
    # border mask.
    M = np.zeros((3, 3, 14, 14), np.float64)
    rng = {0: (0, 13), 1: (0, 14), 2: (1, 14)}
    for kh in range(3):
        for kw in range(3):
            r0, r1 = rng[kh]
            c0, c1 = rng[kw]
            M[kh, kw, r0:r1, c0:c1] = 1.0
    A = np.einsum('ckl,klrs->crs', W2, M) / 196.0   # [1024, 14, 14]

    # Quadrants of the 14x14 concat grid: rows<7,cols<7 = g1(x1);
    # rows>=7,cols<7 = g2(x2); rows<7,cols>=7 = g3(x3 path);
    # rows>=7,cols>=7 = gs(share).  g1/g2/gs weights apply directly to the
    # 7x7 crop windows; only the x3 path needs the scatter (c_w mixes
    # channels with different crop offsets).
    A1 = A[:, 0:7, 0:7].reshape(1024, 49)
    A2 = A[:, 7:14, 0:7].reshape(1024, 49)
    A4 = A[:, 7:14, 7:14].reshape(1024, 49)

    A3 = A[:, 0:7, 7:14]
    Ws3c = np.zeros((1024, 14, 14), np.float64)
    ci = np.arange(1024)[:, None, None]
    ri = (idx_h[2][:, None] + np.arange(7))[:, :, None]
    wi = (idx_w[2][:, None] + np.arange(7))[:, None, :]
    Ws3c[ci, ri, wi] = A3

    # Pull the scattered weights back through the 1x1 conv ...
    Wpool = np.einsum('oc,ohw->chw', c_w, Ws3c)     # [1280, 14, 14]
    # ... and through avg_pool2d(5, stride 2, pad 2) (transposed scatter).
    Ws3 = np.zeros((1280, 28, 28), np.float64)
    for dh in range(-2, 3):
        for dw in range(-2, 3):
            hs = [h for h in range(14) if 0 <= 2 * h + dh < 28]
            ws = [w for w in range(14) if 0 <= 2 * w + dw < 28]
            H = [2 * h + dh for h in hs]
            W_ = [2 * w + dw for w in ws]
            Ws3[:, np.ix_(H, W_)[0], np.ix_(H, W_)[1]] += \
                Wpool[:, np.ix_(hs, ws)[0], np.ix_(hs, ws)[1]] / 25.0

    return (A1.astype(np.float32), A2.astype(np.float32),
            A4.astype(np.float32), Ws3.reshape(1280, 784).astype(np.float32))


def _crop(x, ih, iw):
    """Gather the per-channel 7x7 crop windows: [B,1024,14,14] -> [B,1024,49]."""
    B = x.shape[0]
    ci = np.arange(1024)[:, None, None]
    ri = (ih[:, None] + np.arange(7))[:, :, None]
    wi = (iw[:, None] + np.arange(7))[:, None, :]
    return x[:, ci, ri, wi].reshape(B, 1024, 49).astype(np.float16)


def _act_cols():
    """Batch id for each dense ACT accumulator column."""
    return [blk * BLK + j for blk in ACT_BLOCKS for j in range(BLK)]


def _flat_segs(j):
    """Split batch j's flat range [j*F, (j+1)*F) at CHW boundaries.

    Yields (fs, fe, row, col): batch-local f range -> (psum row, col).
    """
    lo, hi = j * F_TOT, (j + 1) * F_TOT
    while lo < hi:
        nxt = min(hi, (lo // CHW + 1) * CHW)
        yield (lo - j * F_TOT, nxt - j * F_TOT, lo // CHW, lo % CHW)
        lo = nxt


def _build_bass(xbufs=XBUFS):
    nc = bacc.Bacc("TRN2")
    n_act = len(ACT_BLOCKS) * BLK                       # 28
    n_rows = len(PE_BLOCKS) * NCHUNK                    # 81 psum rows
    xin = nc.dram_tensor("xin", [128, NB, F_TOT], _F16, kind="ExternalInput")
    win = nc.dram_tensor("win", [128, F_TOT], _F16, kind="ExternalInput")
    outp = nc.dram_tensor("outp", [n_rows, CHW], _F32, kind="ExternalOutput")
    outa = nc.dram_tensor("outa", [1, n_act], _F32, kind="ExternalOutput")

    with TileContext(nc) as tc:
        with (
            tc.tile_pool(name="xpool", bufs=xbufs) as xpool,
            tc.tile_pool(name="gpool", bufs=3) as gpool,
            tc.tile_pool(name="apool", bufs=1) as apool,
            tc.tile_pool(name="ppool", bufs=1, space="PSUM") as ppool,
        ):
            cpool = apool
            wt = cpool.tile([128, F_TOT], _F16)
            nc.scalar.dma_start(out=wt[:], in_=win[:, :])
            wbb = wt[:].unsqueeze(1).broadcast_to([128, BLK, F_TOT])

            # Sliding ones-column window: z[:, 128] = 1, else 0.  The
            # stationary for psum row r is z[:, 128-r : 256-r] (col r of
            # that window is the ones column).
            z = cpool.tile([128, 256], _F16)
            nc.gpsimd.memset(z[:], 0.0)
            nc.gpsimd.memset(z[:, 128:129], 1.0)
            ones32 = cpool.tile([128, 1], _F32)
            nc.gpsimd.memset(ones32[:], 1.0)

            pchunk = ppool.tile([128, CHW], _F32)       # PE chunk sums
            psa = ppool.tile([1, n_act], _F32)          # ACT batch sums
            acc = apool.tile([128, n_act], _F32)        # ACT accum columns

            first_mm = [True]

            def emit_mm(out_ap, lhsT, rhs, last=False):
                nc.tensor.matmul(out_ap, lhsT=lhsT, rhs=rhs,
                                 start=first_mm[0], stop=last)
                first_mm[0] = False

            pe_i = 0            # dense PE-block index
            act_i = 0           # dense ACT column base
            blocked_i = 0       # physical prod-buffer rotation counter
            for k in range(NBLK):
                single = k in SINGLES
                if single:
                    prods = []
                    for j in range(BLK):
                        xt1 = xpool.tile([128, F_TOT], _F16, tag="xt1")
                        nc.sync.dma_start(
                            out=xt1[:], in_=xin[:, k * BLK + j, :])
                        prod1 = gpool.tile([128, F_TOT], _F16, tag="prod1",
                                           bufs=4)
                        nc.vector.tensor_tensor(
                            prod1[:], xt1[:], wt[:], mybir.AluOpType.mult)
                        prods.append(prod1)
                else:
                    xt = xpool.tile([128, BLK, F_TOT], _F16, tag="xt")
                    nc.sync.dma_start(
                        out=xt[:], in_=xin[:, k * BLK:(k + 1) * BLK, :])
                    prod = gpool.tile([128, BLK * F_TOT + PADF], _F16,
                                      tag="prod", bufs=6)
                    if blocked_i < 6:
                        # zero the 96 flat pad cols once per physical
                        # buffer (6-deep rotation; TT never writes them):
                        # they enter the chunk-8 matmul, and NaNs there
                        # would poison whole psum columns.
                        nc.gpsimd.memset(prod[:, BLK * F_TOT:], 0.0)
                    blocked_i += 1
                    nc.vector.tensor_tensor(
                        prod[:, 0:BLK * F_TOT], xt[:], wbb,
                        mybir.AluOpType.mult)

                if k in PE_BLOCKS:
                    lastk = (k == PE_BLOCKS[-1])
                    if single:
                        # per-batch prod tiles: emit CHW-aligned segment
                        # matmuls that land in the same flat rows/cols as
                        # the blocked chunk layout.
                        segs = [(j,) + s for j in range(BLK)
                                for s in _flat_segs(j)]
                        for i, (j, fs, fe, row, col) in enumerate(segs):
                            r = pe_i * NCHUNK + row
                            emit_mm(pchunk[:, col:col + (fe - fs)],
                                    z[:, 128 - r:256 - r],
                                    prods[j][:, fs:fe],
                                    last=(lastk and i == len(segs) - 1))
                    else:
                        for c in range(NCHUNK):
                            r = pe_i * NCHUNK + c
                            emit_mm(pchunk[:, :],
                                    z[:, 128 - r:256 - r],
                                    prod[:, c * CHW:(c + 1) * CHW],
                                    last=(lastk and c == NCHUNK - 1))
                    pe_i += 1
                else:
                    for j in range(BLK):
                        src = (prods[j][:] if single
                               else prod[:, j * F_TOT:(j + 1) * F_TOT])
                        sink = gpool.tile([128, F_TOT], _F16, tag="sink")
                        nc.scalar.activation(
                            sink[:], src,
                            mybir.ActivationFunctionType.Copy,
                            accum_out=acc[:, act_i:act_i + 1])
                        act_i += 1

            # ACT partition-sum: ones-matmul over the dense accum columns.
            nc.tensor.matmul(psa[:], lhsT=ones32[:], rhs=acc[:],
                             start=True, stop=True)
            resa = apool.tile([1, n_act], _F32)
            nc.vector.tensor_copy(resa[:], psa[:])
            nc.scalar.dma_start(out=outa[:, :], in_=resa[:])

            # PE chunk sums -> SBUF -> DRAM (host finishes the reduction).
            # Sync ring: independent of outa's scalar-ring FIFO.
            resp = apool.tile([n_rows, CHW], _F32)
            nc.vector.tensor_copy(resp[:], pchunk[0:n_rows, :])
            nc.sync.dma_start(out=outp[:, :], in_=resp[:])
    nc.finalize()
    return nc


def _shard_inputs(x1, x2, x3, share_feature, A1, A2, A4, Ws3,
                  idx_h, idx_w):
    """Host-side layout: crop-gather + channel-shard + fp16 cast."""
    x1c = _crop(np.asarray(x1), idx_h[0], idx_w[0])       # [64,1024,49] f16
    x2c = _crop(np.asarray(x2), idx_h[1], idx_w[1])
    shc = _crop(np.asarray(share_feature), idx_h[3], idx_w[3])
    x3h = np.asarray(x3, dtype=np.float16).reshape(NB, 1280 * 784)

    in_maps = []
    for m in range(NCORES):
        cs = slice(m * 128, (m + 1) * 128)
        x3f = x3h[:, m * 128 * F3:(m + 1) * 128 * F3].reshape(NB, 128, F3)
        xin = np.concatenate([
            x1c[:, cs], x2c[:, cs], shc[:, cs], x3f,
            np.zeros((NB, 128, 1), np.float16),
        ], axis=2)                                    # [64, 128, 1128]
        xin = np.ascontiguousarray(xin.transpose(1, 0, 2))  # [128, 64, 1128]

        win = np.concatenate([
            A1[cs], A2[cs], A4[cs],
            Ws3.reshape(-1)[m * 128 * F3:(m + 1) * 128 * F3].reshape(128, F3),
            np.zeros((128, 1), np.float32),
        ], axis=1) * W_SCALE                          # [128, 1128]
        in_maps.append({'xin': xin, 'win': win.astype(np.float16)})
    return in_maps


def _combine(results, lin_b):
    """Sum per-core partials; untangle PE chunk rows; add bias."""
    total = np.zeros(NB, np.float64)
    for r in results:
        p = r['outp'].astype(np.float64)          # [81, 512]
        a = r['outa'].astype(np.float64).reshape(-1)   # [28]
        flat = p.reshape(len(PE_BLOCKS), NCHUNK * CHW)[:, :BLK * F_TOT]
        dots = flat.reshape(len(PE_BLOCKS), BLK, F_TOT).sum(axis=2)
        for i, blk in enumerate(PE_BLOCKS):
            total[blk * BLK:(blk + 1) * BLK] += dots[i]
        for i, b in enumerate(_act_cols()):
            total[b] += a[i]
    return total / W_SCALE + float(lin_b[0])


def _ensure_ntff_hook():
    """Make `trace=True` (e.g. BASS_TRACE=1) work under axon even when the
    image's antenv package lacks axon_hooks: register an equivalent module
    backed by the ctypes NTFF hook from trn_agent_boot."""
    import sys
    import types
    try:
        import antenv.axon_hooks  # noqa: F401
        return
    except Exception:
        pass
    try:
        from trn_agent_boot import trn_boot
        hook = trn_boot._ntff_profile_via_ctypes('/opt/axon/libaxon_pjrt.so')
        mod = types.ModuleType('antenv.axon_hooks')
        mod.get_axon_ntff_profile_hook = lambda: hook
        mod.set_axon_ntff_profile_hook = lambda h: None
        sys.modules['antenv.axon_hooks'] = mod
    except Exception:
        pass


def kernel(x1, x2, x3, share_feature, c_w, conv3d_w, lin_w, lin_b,
           idx_h, idx_w):
    x1, x2, x3 = np.asarray(x1), np.asarray(x2), np.asarray(x3)
    share_feature = np.asarray(share_feature)
    c_w, conv3d_w = np.asarray(c_w), np.asarray(conv3d_w)
    lin_w, lin_b = np.asarray(lin_w), np.asarray(lin_b)
    idx_h, idx_w = np.asarray(idx_h), np.asarray(idx_w)
    _ensure_ntff_hook()
    A1, A2, A4, Ws3 = _build_fold(c_w, conv3d_w, lin_w, idx_h, idx_w)
    in_maps = _shard_inputs(x1, x2, x3, share_feature,
                            A1, A2, A4, Ws3, idx_h, idx_w)
    nc = _build_bass()
    res = run_bass_kernel_spmd(nc, in_maps, core_ids=list(range(NCORES)))
    return _combine(res.results, lin_b).astype(np.float32).reshape(NB, 1)
